# revision 43
# baseline (speedup 1.0000x reference)
"""Multi-head attention layer on 8 TRN2 NeuronCores.

Reference computation (fp32):
    q = query @ Wq + bq; k = key @ Wk + bk; v = value @ Wv + bv
    scores = softmax(q @ k.T / sqrt(64)) per head
    out = (scores @ v) @ Wo + bo

Sharding (tensor-parallel over heads x data-parallel over batch):
core c = 2*b + hh handles batch b and head-half hh (heads hh*8..hh*8+8,
i.e. feature columns hh*512..(hh+1)*512 of Wq/Wk/Wv). Every core computes
q/k/v projections for its feature half over the full sequence, attention
for its 8 heads, and a partial output projection against its 512-row slice
of Wo. The host sums the two partials per batch while unsharding - no
cross-core collectives on device.

On-device layout:
    qT  [512, L]  = Wq_h.T @ xqT        (feature-major)
    kT  [512, L]  = Wk_h.T @ xkT
    v   [L, 512]  = xvT.T @ Wv_h + 1s*bv (Lk-major, per-head 66-col strips,
                                          col 64 = ones for softmax sums)
    sT  [Lk, Lq]  = kT_h.T @ qT_h        (per head, K=64)
    eT  = exp(sT / 8)                    (ScalarE; no max-subtract: |sT/8|<~4)
    o_aug [Lq 128-tile, 65] = eT.T @ v_aug  (transposed AV: out partition =
                  Lq, free = 65; col 64 = softmax sums per Lq row -> exact
                  per-partition reciprocal + tensor_scalar normalize, no
                  cross-partition broadcast needed)
    o2  [Lq, 128] = normalized head pair -> PE transpose (identity matmul)
                  -> oT [128 feat, Lq 128] -> oT_all
    outT_partial [1024, L] = Wo_h.T @ oT_all (+ bo on hh=0 cores only)
Host: out[b] = (outT_partial[2b] + outT_partial[2b+1]).T

Why transposed AV: PE cost is (output free size) x (K-accum steps); the
[65, Lq] orientation wastes half the array (65 of 128 output partitions),
[Lq, 65] is full-width (54.6us vs 109us per core on the AV term).

Schedule: 16 groups g = oct*8 + head (oct = Lq half of 1024). Per group:
16 score tiles [128 Lk, 1024 Lq] (2x N=512 matmuls into a dedicated
2-buf PSUM pool so the next tile's matmuls always overlap the current
exp), each followed by exp on ScalarE into retained bf16 e tiles. The
attn-V of group g-1 (8 Lq tiles x 16 Lk accum steps into 1-bank PSUM
accumulators), projection sub-units, and the output projection are
sprinkled into fixed slots between score tiles so PE tracks just behind
ScalarE (~267us of exp). SBUF is tight, so x activations arrive
just-in-time: the host lays each projection sub-unit's x slice out
contiguously ([128, KT, 512] per (proj, L-half, 512-col n)) and each is
DMA'd into a 4-buf ring one group ahead of its single consumer.
PSUM: scores 2x2 banks + proj/transpose 2x2 banks + 2x 1-bank o
accumulators = 8 banks.
"""

import numpy as np
import ml_dtypes

import concourse.bacc as bacc
import concourse.bass as bass
import concourse.mybir as mybir
import concourse.tile as tile
from concourse import bass_utils

B, L, DIM = 4, 2048, 1024
H, HD = 16, 64
N_CORES = 8
HL = 8             # local heads per core
FD = 512           # local feature columns (8 heads * 64)
KT = DIM // 128    # 8 contraction k-tiles for projections
MT = FD // 128     # 4 output feature tiles for q/k/v projections
NLK = L // 128     # 16 Lk tiles
VSTR = 66          # per-head stride in v_sb (64 vals + ones col + pad)

BF16 = mybir.dt.bfloat16
F32 = mybir.dt.float32
AF = mybir.ActivationFunctionType


def _build_body(tc, io):
    nc = tc.nc
    xq, xk, xv, wq, wk, wv, wo, bq, bk, bo, bvr, ident, outT = io

    from contextlib import ExitStack
    with ExitStack() as ctx:
        const = ctx.enter_context(tc.tile_pool(name="const", bufs=1))
        wpool = ctx.enter_context(tc.tile_pool(name="wpool", bufs=1))
        xqk_pool = ctx.enter_context(tc.tile_pool(name="xqk", bufs=4))
        vx_pool = ctx.enter_context(tc.tile_pool(name="vx", bufs=2))
        qk_sb = ctx.enter_context(tc.tile_pool(name="qk_sb", bufs=1))
        e_pool = ctx.enter_context(tc.tile_pool(name="e_pool", bufs=32))
        o2_pool = ctx.enter_context(tc.tile_pool(name="o2_pool", bufs=18))
        small = ctx.enter_context(tc.tile_pool(name="small", bufs=8))
        av_stage = ctx.enter_context(tc.tile_pool(name="av_stage", bufs=4))
        stage = ctx.enter_context(tc.tile_pool(name="stage", bufs=4))
        # PSUM (8 banks): scores 3x 2-bank (the third buffer absorbs
        # DVE-exp queue jitter) + 2x 1-bank shared by attn-V accumulators,
        # projection halves, and transposes.
        s_ps_pool = ctx.enter_context(
            tc.tile_pool(name="s_ps", bufs=3, space="PSUM"))
        o_ps_pool = ctx.enter_context(
            tc.tile_pool(name="o_ps", bufs=2, space="PSUM"))

        # ---- constants (tiles now; DMAs ordered inside the prologue) ----
        bq_sb = const.tile([128, MT], F32)
        bk_sb = const.tile([128, MT], F32)
        bo_sb = const.tile([128, KT], F32)
        bv_bc = const.tile([128, FD], BF16)
        id_sb = const.tile([128, 128], BF16)

        # ---- persistent activations ----
        qT = qk_sb.tile([128, MT, L], BF16)
        kTt = qk_sb.tile([128, MT, L], BF16)
        v_sb = qk_sb.tile([128, NLK, HL * VSTR], BF16)
        oT_all = qk_sb.tile([128, MT, L], BF16)

        # ones column of v_aug (written once; proj copies fill the rest)
        for h in range(HL):
            nc.vector.memset(v_sb[:, :, h * VSTR + 64:h * VSTR + 65], 1.0)

        # ---- weights (8KB/partition each; wv's tile is reused for wo,
        # which is only needed after the last vproj) ----
        wq_sb = wpool.tile([128, MT, KT, 128], BF16, tag="wq")
        wk_sb = wpool.tile([128, MT, KT, 128], BF16, tag="wk")
        wv_sb = wpool.tile([128, KT, FD], BF16, tag="wv")
        wo_sb = wv_sb.rearrange("p a b -> p (a b)").rearrange(
            "p (c d) -> p c d", d=DIM)

        # ---- just-in-time x slices ----
        x_store = {}

        def load_qk(uid, which, half, n):
            src = {"q": xq, "k": xk}[which]
            t = xqk_pool.tile([128, KT, 512], BF16, tag="xqk",
                              name=f"x_{uid}_{n}")
            nc.sync.dma_start(out=t, in_=src[half * 2 + n])
            x_store[(uid, n)] = t

        def load_v(half, j):
            t = vx_pool.tile([128, KT, 256], BF16, tag="vx",
                             name=f"xv_{half}_{j}")
            nc.sync.dma_start(out=t, in_=xv[half * 4 + j])
            x_store[("v", half, j)] = t

        # ---- projection / output-projection units ----
        def qk_half(uid, which, mt, half, n):
            """Half (512 cols) of a q/k projection unit on the o_ps ring;
            the bias-evac runs on ScalarE (slack engine) so the DVE queue
            stays short for latency-critical ring releases."""
            w_sb, dst, b_sb = ((wq_sb, qT, bq_sb) if which == "q"
                               else (wk_sb, kTt, bk_sb))
            xs = x_store.pop((uid, n))
            ps = o_ps_pool.tile([128, 512], F32, tag="o",
                                name=f"psh_{uid}_{n}")
            for kt in range(KT):
                nc.tensor.matmul(
                    ps, w_sb[:, mt, kt, :], xs[:, kt, :],
                    start=(kt == 0), stop=(kt == KT - 1))
            c0 = half * 1024 + n * 512
            nc.scalar.activation(
                dst[:, mt, c0:c0 + 512], ps, AF.Identity,
                bias=b_sb[:, mt:mt + 1])

        def qk_run(uid, which, mt, half):
            qk_half(uid, which, mt, half, 0)
            qk_half(uid, which, mt, half, 1)

        def vp_half(half, j, r2):
            """One Lk-tile (half*8 + 2j + r2) of the v projection; bias
            added by the DVE evac against the pre-broadcast bv tile."""
            xs = x_store[("v", half, j)]
            rt = half * 8 + 2 * j + r2
            ps_v = o_ps_pool.tile([128, 512], F32, tag="o",
                                  name=f"psv_{rt}")
            for kt in range(KT):
                nc.tensor.matmul(
                    ps_v, xs[:, kt, r2 * 128:(r2 + 1) * 128],
                    wv_sb[:, kt, 0:FD],
                    start=(kt == 0), stop=(kt == KT - 1))
            dst = v_sb[:, rt, :].rearrange(
                "p (h d) -> p h d", d=VSTR)[:, :, 0:64]
            nc.vector.tensor_tensor(
                out=dst,
                in0=ps_v.rearrange("p (h d) -> p h d", d=64),
                in1=bv_bc.rearrange("p (h d) -> p h d", d=64),
                op=mybir.AluOpType.add)
            if r2 == 1:
                x_store.pop(("v", half, j))

        def oproj_half(lqh, mt, n2):
            """One 512-col half of the partial output projection
            outT = Wo_h.T @ oT_all (+ bo), pipelined on the o_ps ring."""
            n = lqh * 2 + n2
            ps_o = o_ps_pool.tile([128, 512], F32, tag="o",
                                  name=f"psoh_{mt}_{n2}")
            for kt in range(MT):
                nc.tensor.matmul(
                    ps_o, wo_sb[:, kt, mt * 128:(mt + 1) * 128],
                    oT_all[:, kt, n * 512:(n + 1) * 512],
                    start=(kt == 0), stop=(kt == MT - 1))
            st = stage.tile([128, 512], F32, tag="stage")
            nc.vector.tensor_scalar(
                out=st, in0=ps_o, scalar1=bo_sb[:, mt:mt + 1],
                scalar2=None, op0=mybir.AluOpType.add)
            nc.sync.dma_start(
                out=outT[mt * 128:(mt + 1) * 128, n * 512:(n + 1) * 512],
                in_=st)

        def oproj_unit(lqh, mt):
            oproj_half(lqh, mt, 0)
            oproj_half(lqh, mt, 1)

        # ---- attention pieces ----
        e_tiles = {}    # g -> list of 16 e tiles
        o2_tiles = {}   # (oct, pair, lq) -> o2 stage tile

        # Schraudolph bit-trick exp for the DVE-offloaded score tiles:
        # bf16_bits(exp(s/8)) ~ int16(s * (2^7/ln2)/8 + (127*2^7 - 5.8)).
        # ~2% rms per-element error on 25% of tiles -> ~1.1e-2 output error
        # (vs the 2e-2 gate); frees ScalarE, the pacing engine.
        SCH_A = (2.0 ** 7) / float(np.log(2.0)) / 8.0
        SCH_B = 127.0 * 128.0 - 5.8
        SCH_LKT = (3, 7, 11, 15)

        def score_tile(g, lkt):
            oct_, h = g // 8, g % 8
            mt, hp = h // 2, (h % 2) * 64
            q0 = oct_ * 1024
            s_ps = s_ps_pool.tile([128, 1024], F32, tag="s", name="s_ps")
            for n in range(2):
                nc.tensor.matmul(
                    s_ps[:, n * 512:(n + 1) * 512],
                    kTt[hp:hp + 64, mt, lkt * 128:(lkt + 1) * 128],
                    qT[hp:hp + 64, mt, q0 + n * 512:q0 + (n + 1) * 512],
                    start=True, stop=True)
            e_t = e_pool.tile([128, 1024], BF16, tag="e",
                              name=f"e_{g}_{lkt}")
            if lkt in SCH_LKT:
                # two halves so a queued ring-release copy waits at most
                # ~0.6us behind the exp in the in-order DVE queue
                for nh in range(2):
                    nc.vector.tensor_scalar(
                        out=e_t.bitcast(mybir.dt.int16)[:, nh * 512:
                                                        (nh + 1) * 512],
                        in0=s_ps[:, nh * 512:(nh + 1) * 512],
                        scalar1=SCH_A, scalar2=SCH_B,
                        op0=mybir.AluOpType.mult, op1=mybir.AluOpType.add)
            else:
                nc.scalar.activation(e_t, s_ps, AF.Exp, scale=0.125)
            e_tiles.setdefault(g, []).append(e_t)

        def av_unit(g, lq):
            """Attn-V for one Lq tile of group g: 16 Lk accum steps, then
            normalize into the o2 stage; transpose on pair completion."""
            oct_, h = g // 8, g % 8
            pair, side = h // 2, h % 2
            es = e_tiles[g]
            glq = oct_ * 8 + lq
            o_ps = o_ps_pool.tile([128, 512], F32, tag="o",
                                  name=f"o_{g}_{lq}")
            for lkt in range(NLK):
                nc.tensor.matmul(
                    o_ps[:, 0:65],
                    es[lkt][:, lq * 128:(lq + 1) * 128],
                    v_sb[:, lkt, h * VSTR:h * VSTR + 65],
                    start=(lkt == 0), stop=(lkt == NLK - 1))
            # one fast copy releases the PSUM bank; normalize runs off-ring
            stg = av_stage.tile([128, 65], F32, tag="avs",
                                name=f"avst_{g}_{lq}")
            nc.vector.tensor_copy(out=stg, in_=o_ps[:, 0:65])
            # exact reciprocal of softmax sums (col 64 = one per partition)
            rec = small.tile([128, 1], F32, tag="rec")
            nc.vector.reciprocal(out=rec, in_=stg[:, 64:65])
            if side == 0:
                o2 = o2_pool.tile([128, 128], BF16, tag="o2",
                                  name=f"o2_{oct_}_{pair}_{lq}")
                o2_tiles[(oct_, pair, lq)] = o2
            else:
                o2 = o2_tiles[(oct_, pair, lq)]
            nc.vector.tensor_scalar(
                out=o2[:, side * 64:side * 64 + 64], in0=stg[:, 0:64],
                scalar1=rec, scalar2=None, op0=mybir.AluOpType.mult)
            if side == 1:
                # pair complete for this lq: transpose [Lq,128] -> [128,Lq]
                tr = o_ps_pool.tile([128, 128], BF16, tag="o",
                                    name=f"tr_{oct_}_{pair}_{lq}")
                nc.tensor.transpose(tr, o2, id_sb)
                nc.vector.tensor_copy(
                    out=oT_all[:, pair, glq * 128:(glq + 1) * 128],
                    in_=tr)
                del o2_tiles[(oct_, pair, lq)]
            if lq == 7:
                del e_tiles[g]

        # ---- emission schedule ----
        # Prologue: DMA emission order = shared-DMA-device service order, so
        # order strictly by first need: wk+xk(n0) -> wq+xq -> xk(n1) ->
        # k01's x -> wv -> first v slices. First exp fires ~16us in.
        nc.sync.dma_start(out=wk_sb[:, 0], in_=wk[0])
        load_qk("k00", "k", 0, 0)
        nc.sync.dma_start(out=wq_sb[:, 0], in_=wq[0])
        load_qk("q00", "q", 0, 0)
        nc.sync.dma_start(out=bk_sb, in_=bk)
        nc.sync.dma_start(out=bq_sb, in_=bq)
        load_qk("q00", "q", 0, 1)
        load_qk("k00", "k", 0, 1)
        load_qk("k01", "k", 1, 0)
        load_qk("k01", "k", 1, 1)
        nc.sync.dma_start(out=wv_sb, in_=wv)
        load_v(0, 0)
        load_v(0, 1)
        nc.sync.dma_start(out=bv_bc, in_=bvr)
        nc.sync.dma_start(out=id_sb, in_=ident)
        nc.sync.dma_start(out=bo_sb, in_=bo)
        for _mt in range(1, MT):
            nc.sync.dma_start(out=wk_sb[:, _mt], in_=wk[_mt])
            nc.sync.dma_start(out=wq_sb[:, _mt], in_=wq[_mt])
        qk_half("k00", "k", 0, 0, 0)
        qk_half("q00", "q", 0, 0, 0)
        qk_half("q00", "q", 0, 0, 1)

        # slot[g][i] = thunks emitted right after score tile i of group g
        # (-1 = before the group's first score tile). Loads sit ~4 slots
        # ahead of their single consumer; the 4-buf x ring makes this safe.
        QK, QH, VP, OP, OPH = qk_run, qk_half, vp_half, oproj_unit, oproj_half
        LQ, LV = load_qk, load_v

        def TH(f, *a):
            return lambda: f(*a)

        slots = {g: {} for g in range(16)}

        def put(g, i, *thunks):
            slots[g].setdefault(i, []).extend(thunks)

        # g0: rest of mt0 (k cols 512:1024 then 1024:2048) + v half-0
        put(0, 1, TH(QH, "k00", "k", 0, 0, 1))
        put(0, 2, TH(LV, 0, 2))
        put(0, 3, TH(LV, 0, 3))
        put(0, 4, TH(LV, 1, 0))
        put(0, 5, TH(LV, 1, 1))
        put(0, 6, TH(QK, "k01", "k", 0, 1))
        put(0, 8, TH(VP, 0, 0, 0), TH(LV, 1, 2))
        put(0, 9, TH(VP, 0, 0, 1), TH(LV, 1, 3))
        put(0, 10, TH(VP, 0, 1, 0))
        put(0, 11, TH(VP, 0, 1, 1))
        put(0, 12, TH(VP, 0, 2, 0))
        put(0, 13, TH(VP, 0, 2, 1))
        put(0, 14, TH(VP, 0, 3, 0))
        put(0, 15, TH(VP, 0, 3, 1))
        # g1: v half-1 projections, then av(0) (gated on full v)
        put(1, 0, TH(VP, 1, 0, 0))
        put(1, 1, TH(VP, 1, 0, 1))
        put(1, 2, TH(VP, 1, 1, 0))
        put(1, 3, TH(VP, 1, 1, 1))
        put(1, 4, TH(VP, 1, 2, 0))
        put(1, 5, TH(VP, 1, 2, 1))
        put(1, 6, TH(VP, 1, 3, 0))
        put(1, 7, TH(VP, 1, 3, 1))
        put(1, 8, TH(LQ, "k10", "k", 0, 0), TH(LQ, "k10", "k", 0, 1))
        put(1, 10, TH(LQ, "q10", "q", 0, 0), TH(LQ, "q10", "q", 0, 1))
        # g2: mt1 projections for h2/h3 (before the first score tile)
        put(2, -1, TH(QK, "k10", "k", 1, 0), TH(QK, "q10", "q", 1, 0))
        put(2, 0, TH(LQ, "k11", "k", 1, 0), TH(LQ, "k11", "k", 1, 1))
        put(2, 6, TH(QK, "k11", "k", 1, 1))
        # g3: prefetch mt2; run its units late in the group
        put(3, 0, TH(LQ, "k20", "k", 0, 0), TH(LQ, "k20", "k", 0, 1))
        put(3, 2, TH(LQ, "q20", "q", 0, 0), TH(LQ, "q20", "q", 0, 1))
        put(3, 12, TH(QK, "k20", "k", 2, 0))
        put(3, 14, TH(QK, "q20", "q", 2, 0))
        # g4: mt2 half1 for h4/h5
        put(4, 0, TH(LQ, "k21", "k", 1, 0), TH(LQ, "k21", "k", 1, 1))
        put(4, 6, TH(QK, "k21", "k", 2, 1))
        put(4, 9, TH(LQ, "q01", "q", 1, 0), TH(LQ, "q01", "q", 1, 1))
        # g5: oct1 q for mt0; prefetch + run mt3 late
        put(5, 2, TH(QK, "q01", "q", 0, 1))
        put(5, 4, TH(LQ, "k30", "k", 0, 0), TH(LQ, "k30", "k", 0, 1))
        put(5, 6, TH(LQ, "q30", "q", 0, 0), TH(LQ, "q30", "q", 0, 1))
        put(5, 12, TH(QK, "k30", "k", 3, 0))
        # g6: mt3 for h6/h7
        put(6, -1, TH(QK, "q30", "q", 3, 0))
        put(6, 0, TH(LQ, "k31", "k", 1, 0), TH(LQ, "k31", "k", 1, 1))
        put(6, 6, TH(QK, "k31", "k", 3, 1))
        put(6, 9, TH(LQ, "q11", "q", 1, 0), TH(LQ, "q11", "q", 1, 1))
        # g7+: oct1 q columns; wo load reuses wv's tile (vproj long done)
        put(7, 2, TH(QK, "q11", "q", 1, 1))
        put(7, 4, lambda: nc.sync.dma_start(out=wo_sb, in_=wo))
        put(7, 6, TH(LQ, "q21", "q", 1, 0), TH(LQ, "q21", "q", 1, 1))
        put(8, 2, TH(QK, "q21", "q", 2, 1))
        put(8, 6, TH(LQ, "q31", "q", 1, 0), TH(LQ, "q31", "q", 1, 1))
        put(9, 2, TH(QK, "q31", "q", 3, 1))
        # oct0 output projection (oT_all cols 0:1024 complete after av(7)
        # inside g8), spread over g9..g15
        put(9, 8, TH(OP, 0, 0))
        put(10, 4, TH(OP, 0, 1))
        put(11, 4, TH(OP, 0, 2))
        put(12, 4, TH(OP, 0, 3))
        put(12, 10, TH(OP, 0, 4))
        put(13, 4, TH(OP, 0, 5))
        put(14, 4, TH(OP, 0, 6))
        put(15, 4, TH(OP, 0, 7))

        for g in range(16):
            avs = [TH(av_unit, g - 1, lq) for lq in range(8)] if g else []
            # in g1 the avs must follow the vproj units (full-Lk accum)
            av_from = 10 if g == 1 else 0
            for th in slots[g].get(-1, ()):
                th()
            for lkt in range(NLK):
                score_tile(g, lkt)
                for th in slots[g].get(lkt, ()):
                    th()
                if avs and lkt >= av_from and lkt % 2 == 0:
                    avs.pop(0)()
            for a in avs:
                a()
        # tail: last group's attn-V interleaved with the pipelined halves
        # of the oct1 output projection (n2=0 needs av(15) lq 0..3 only)
        for lq in range(4):
            av_unit(15, lq)
        for mt in range(KT // 2):
            oproj_half(1, mt, 0)
        for lq in range(4, 8):
            av_unit(15, lq)
        for mt in range(KT // 2, KT):
            oproj_half(1, mt, 0)
        for mt in range(KT):
            oproj_half(1, mt, 1)


_CACHED = {}


def _get_nc():
    if "nc" not in _CACHED:
        nc = bacc.Bacc("TRN2", target_bir_lowering=False, debug=False)
        io = (
            # x slices pre-laid by the host so each projection sub-unit's
            # input is one contiguous [128, KT, cols] DMA; leading dim =
            # flat 512-col (qk) / 256-col (v) chunk of the sequence
            nc.dram_tensor("xq", [4, 128, KT, 512], BF16,
                           kind="ExternalInput").ap(),
            nc.dram_tensor("xk", [4, 128, KT, 512], BF16,
                           kind="ExternalInput").ap(),
            nc.dram_tensor("xv", [8, 128, KT, 256], BF16,
                           kind="ExternalInput").ap(),
            nc.dram_tensor("wq", [MT, 128, KT, 128], BF16,
                           kind="ExternalInput").ap(),
            nc.dram_tensor("wk", [MT, 128, KT, 128], BF16,
                           kind="ExternalInput").ap(),
            nc.dram_tensor("wv", [128, KT, FD], BF16,
                           kind="ExternalInput").ap(),
            nc.dram_tensor("wo", [128, MT, DIM], BF16,
                           kind="ExternalInput").ap(),
            nc.dram_tensor("bq", [128, MT], F32, kind="ExternalInput").ap(),
            nc.dram_tensor("bk", [128, MT], F32, kind="ExternalInput").ap(),
            nc.dram_tensor("bo", [128, KT], F32, kind="ExternalInput").ap(),
            nc.dram_tensor("bvr", [128, FD], BF16,
                           kind="ExternalInput").ap(),
            nc.dram_tensor("ident", [128, 128], BF16,
                           kind="ExternalInput").ap(),
            nc.dram_tensor("outT", [DIM, L], F32, kind="ExternalOutput").ap(),
        )
        with tile.TileContext(nc) as tc:
            _build_body(tc, io)
        nc.compile()
        _CACHED["nc"] = nc
    return _CACHED["nc"]


def _prep_maps(query, key, value, Wq, bq, Wk, bk, Wv, bv, Wo, bo):
    bf = ml_dtypes.bfloat16
    f32 = np.float32

    xqk = {}
    xvv = {}
    for name, arr in (("q", query), ("k", key), ("v", value)):
        for b_idx in range(B):
            xt = np.ascontiguousarray(arr[b_idx].T.astype(bf))  # [1024, L]
            if name == "v":
                # [kt, p, c, 256] -> [c, p, kt, 256]
                a = xt.reshape(KT, 128, 8, 256)
                xvv[b_idx] = np.ascontiguousarray(a.transpose(2, 1, 0, 3))
            else:
                # [kt, p, c, 512] -> [c, p, kt, 512]
                a = xt.reshape(KT, 128, 4, 512)
                xqk[(name, b_idx)] = np.ascontiguousarray(
                    a.transpose(2, 1, 0, 3))

    ident = np.eye(128, dtype=np.float32).astype(bf)

    halves = []
    for hh in range(2):
        cols = slice(hh * FD, (hh + 1) * FD)
        halves.append({
            "wq": np.ascontiguousarray(
                Wq[:, cols].astype(bf).reshape(KT, 128, MT, 128).transpose(
                    2, 1, 0, 3)),
            "wk": np.ascontiguousarray(
                Wk[:, cols].astype(bf).reshape(KT, 128, MT, 128).transpose(
                    2, 1, 0, 3)),
            "wv": np.ascontiguousarray(
                Wv[:, cols].astype(bf).reshape(KT, 128, FD).transpose(
                    1, 0, 2)),
            "wo": np.ascontiguousarray(
                Wo[cols, :].astype(bf).reshape(MT, 128, DIM).transpose(
                    1, 0, 2)),
            "bq": np.ascontiguousarray(
                np.asarray(bq, f32)[cols].reshape(MT, 128).T),
            "bk": np.ascontiguousarray(
                np.asarray(bk, f32)[cols].reshape(MT, 128).T),
            "bvr": np.ascontiguousarray(np.broadcast_to(
                np.asarray(bv, f32)[cols].astype(bf).reshape(1, FD),
                (128, FD))),
            # bo applied once (on the hh=0 partial)
            "bo": np.ascontiguousarray(
                (np.asarray(bo, f32) if hh == 0 else
                 np.zeros(DIM, f32)).reshape(KT, 128).T),
            "ident": ident,
        })
    in_maps = []
    for c in range(N_CORES):
        b_idx, hh = c // 2, c % 2
        in_maps.append(dict(
            halves[hh],
            xq=xqk[("q", b_idx)], xk=xqk[("k", b_idx)], xv=xvv[b_idx],
        ))
    return in_maps


def kernel(query, key, value, Wq, bq, Wk, bk, Wv, bv, Wo, bo, **run_kwargs):
    query = np.asarray(query, np.float32)
    key = np.asarray(key, np.float32)
    value = np.asarray(value, np.float32)
    Wq, Wk, Wv, Wo = (np.asarray(w, np.float32) for w in (Wq, Wk, Wv, Wo))
    bq, bk, bv, bo = (np.asarray(b, np.float32) for b in (bq, bk, bv, bo))
    nc = _get_nc()
    in_maps = _prep_maps(query, key, value, Wq, bq, Wk, bk, Wv, bv, Wo, bo)
    res = bass_utils.run_bass_kernel_spmd(
        nc, in_maps, core_ids=list(range(N_CORES)), **run_kwargs)
    out = np.empty((B, L, DIM), np.float32)
    for b_idx in range(B):
        pa = res.results[2 * b_idx]["outT"]
        pb = res.results[2 * b_idx + 1]["outT"]
        out[b_idx] = (pa + pb).T
    _CACHED["last_results"] = res
    return out
